# revision 1
# baseline (speedup 1.0000x reference)
"""Trainium2 Bass kernel for nn_CASAtt_MultiHead_v1 (CAS attention block).

Reference computation (per sample):
    qkv = 1x1 conv (qkv_w) -> q, k, v                        [512, 56, 56] each
    q <- SE(dwconv3x3(q, sq_w, sq_b))   (per-head squeeze-excite)
    k <- SE(dwconv3x3(k, sk_w, sk_b))
    out = proj(dwconv3x3(q + k, dwc_w, dwc_b) * v) + proj_b + x

Distribution: data-parallel over batch, 2 samples per NeuronCore x 8 cores.

Layout: channels on partitions, 4 chunks of 128 (chunk == SE head).
GEMMs run in bf16 (fp32 PSUM accumulate); fp32 matmuls on trn2 run in
LOW_HIGH mode at ~3x the cost, so everything streaming through the PE is
bf16.  Depthwise convs: 9 taps, applied either as diagonal-matrix matmuls
accumulated in PSUM (TensorE), or as fused scalar-MAC chains
(scalar_tensor_tensor) on VectorE over *contiguous* padded-flat slices so
the DVE 2x bf16 perf mode engages where alignment allows (strided views
drop it to 1x).  The conv domain is zero-padded HPxWP with WP=59 so that
5 of 9 tap offsets have even element parity (bf16 4-byte alignment for
the DVE perf mode).  Conv outputs computed over full padded rows produce
garbage only in pad columns, which are never read.  SE average-pool is
fused into the conv1 drain via accum_out.  Since depthwise conv and the
SE scale are per-channel linear, m = s_q*dwq + s_k*dwk is built after
both branches and a single third conv runs on m.  Mixed-dtype
tensor-tensor DVE ops (psum f32 + bf16 operand) produce NaN on hardware
(fine in CoreSim) -- every tensor-tensor-class op here keeps both tensor
operands the same dtype.
"""

import numpy as np

DIM = 512
NH = 4
HD = 128
HD4 = 32
B, H_FULL, W = 16, 56, 56
N_CORES = 8

TAPS = [(dy, dx) for dy in (-1, 0, 1) for dx in (-1, 0, 1)]


def default_cfg():
    return dict(
        b_local=B // N_CORES,
        H=H_FULL,
        rows_per_tile=8,
        conv_bf16=True,
        gemm_bf16=True,
        # engine per (branch, oc) for conv1:  'pe' | 'dve'
        conv1_assign={(br, oc): 'pe' for br in (0, 1) for oc in range(NH)},
        repeat=1,
    )


def build_nc(cfg):
    """Build + compile the Bacc program for one core (SPMD across 8)."""
    import concourse.bass as bass
    import concourse.mybir as mybir
    import concourse.tile as tile
    from concourse import bacc
    from contextlib import ExitStack

    f32 = mybir.dt.float32
    bf16 = mybir.dt.bfloat16
    cdt = bf16 if cfg['conv_bf16'] else f32
    gdt = bf16 if cfg['gemm_bf16'] else f32

    BL = cfg['b_local']
    H = cfg['H']
    TH = cfg['rows_per_tile']
    NT = H // TH
    assert NT * TH == H
    TN = TH * W
    HP, WP = H + 2, W + 2
    PADN = HP * WP
    TPAD = TH * WP
    AF = mybir.ActivationFunctionType
    AL = mybir.AluOpType
    # DVE tap order: even-parity offsets first (2x mode), odd-parity last;
    # the final op is strided (1x regardless), so give it an odd tap
    evens = [j for j, (dy, dx) in enumerate(TAPS) if (dy * WP + dx) % 2 == 0]
    odds = [j for j, (dy, dx) in enumerate(TAPS) if (dy * WP + dx) % 2]
    dve_tap_order = evens + odds if odds else list(range(9))

    nc = bacc.Bacc("TRN2", target_bir_lowering=False, debug=False,
                   enable_asserts=False, num_devices=N_CORES)

    # ---------------- DRAM I/O ----------------
    x_d = nc.dram_tensor("x", [BL, DIM, H, W], gdt, kind="ExternalInput").ap()
    out_d = nc.dram_tensor("out", [BL, DIM, H, W], f32, kind="ExternalOutput").ap()
    wq_d = nc.dram_tensor("wq_t", [DIM, DIM], gdt, kind="ExternalInput").ap()
    wk_d = nc.dram_tensor("wk_t", [DIM, DIM], gdt, kind="ExternalInput").ap()
    wv_d = nc.dram_tensor("wv_t", [DIM, DIM], gdt, kind="ExternalInput").ap()
    wp_d = nc.dram_tensor("proj_t", [DIM, DIM], gdt, kind="ExternalInput").ap()
    dg1_d = [nc.dram_tensor(n, [NH, 9, HD, HD], cdt, kind="ExternalInput").ap()
             for n in ("diag1q", "diag1k")]
    dg2_d = nc.dram_tensor("diag2", [NH, 9, HD, HD], cdt, kind="ExternalInput").ap()
    wv1_d = [nc.dram_tensor(n, [NH, HD, 9], f32, kind="ExternalInput").ap()
             for n in ("wvec1q", "wvec1k")]
    b1_d = [nc.dram_tensor(n, [DIM, 1], f32, kind="ExternalInput").ap()
            for n in ("sq_b", "sk_b")]
    dwcb_d = nc.dram_tensor("dwc_b", [DIM, 1], f32, kind="ExternalInput").ap()
    projb_d = nc.dram_tensor("proj_b", [DIM, 1], f32, kind="ExternalInput").ap()
    sew1_d = [nc.dram_tensor(n, [NH, HD, HD4], f32, kind="ExternalInput").ap()
              for n in ("se_w1q", "se_w1k")]
    seb1_d = [nc.dram_tensor(n, [NH, HD4, 1], f32, kind="ExternalInput").ap()
              for n in ("se_b1q", "se_b1k")]
    sew2_d = [nc.dram_tensor(n, [NH, HD4, HD], f32, kind="ExternalInput").ap()
              for n in ("se_w2q", "se_w2k")]
    seb2_d = [nc.dram_tensor(n, [NH, HD, 1], f32, kind="ExternalInput").ap()
              for n in ("se_b2q", "se_b2k")]

    with tile.TileContext(nc) as tc, ExitStack() as ctx:
        const = ctx.enter_context(tc.tile_pool(name="const", bufs=1))
        big = ctx.enter_context(tc.tile_pool(name="big", bufs=1))
        wpool = ctx.enter_context(tc.tile_pool(name="wpool", bufs=1))
        xpool = ctx.enter_context(tc.tile_pool(name="xpool", bufs=2))
        vpool = ctx.enter_context(tc.tile_pool(name="vpool", bufs=2))
        o2pool = ctx.enter_context(tc.tile_pool(name="o2pool", bufs=2))
        otpool = ctx.enter_context(tc.tile_pool(name="otpool", bufs=2))
        dgpool = ctx.enter_context(tc.tile_pool(name="dgpool", bufs=1))
        statpool = ctx.enter_context(tc.tile_pool(name="statpool", bufs=2))
        mmpool = ctx.enter_context(tc.tile_pool(name="mmpool", bufs=5, space="PSUM"))
        sepool = ctx.enter_context(tc.tile_pool(name="sepool", bufs=2, space="PSUM"))

        # ---------- persistent SBUF ----------
        # padded conv-domain buffers; 2-elem slop so padded-space tap reads
        # (offsets -WP-1 .. +WP+1) stay in bounds
        qpad = [big.tile([HD, PADN + 2], cdt, name=f"qpad{c}") for c in range(NH)]
        kpad = [big.tile([HD, PADN + 2], cdt, name=f"kpad{c}") for c in range(NH)]
        dwq = [big.tile([HD, PADN + 2], cdt, name=f"dwq{c}") for c in range(NH)]
        dwk = [big.tile([HD, PADN + 2], cdt, name=f"dwk{c}") for c in range(NH)]

        def pad3(t):
            return t[:, 1:1 + PADN].rearrange("p (h w) -> p h w", w=WP)

        qpad3, kpad3 = [pad3(t) for t in qpad], [pad3(t) for t in kpad]
        dwq3, dwk3 = [pad3(t) for t in dwq], [pad3(t) for t in dwk]

        # persistent DVE-conv accumulator rotation buffers (pad cells may hold
        # stale garbage between uses; only interior cells are ever consumed)
        acc_g = [big.tile([HD, PADN + 2], cdt, name=f"accg{i}") for i in range(3)]

        for tt in qpad + kpad + dwq + dwk + acc_g:
            nc.vector.memset(tt, 0.0)

        # small constants
        bias1 = [[const.tile([HD, 1], f32, name=f"b1_{br}_{c}") for c in range(NH)]
                 for br in range(2)]
        dwcb = [const.tile([HD, 1], f32, name=f"dwcb{c}") for c in range(NH)]
        projb = [const.tile([HD, 1], f32, name=f"projb{c}") for c in range(NH)]
        for c in range(NH):
            sl = slice(c * HD, (c + 1) * HD)
            for br in range(2):
                nc.sync.dma_start(bias1[br][c], b1_d[br][sl])
            nc.sync.dma_start(dwcb[c], dwcb_d[sl])
            nc.sync.dma_start(projb[c], projb_d[sl])
        sew1 = [[const.tile([HD, HD4], f32, name=f"sew1_{br}_{c}") for c in range(NH)]
                for br in range(2)]
        seb1 = [[const.tile([HD4, 1], f32, name=f"seb1_{br}_{c}") for c in range(NH)]
                for br in range(2)]
        sew2 = [[const.tile([HD4, HD], f32, name=f"sew2_{br}_{c}") for c in range(NH)]
                for br in range(2)]
        seb2 = [[const.tile([HD, 1], f32, name=f"seb2_{br}_{c}") for c in range(NH)]
                for br in range(2)]
        wvec1 = [[const.tile([HD, 9], f32, name=f"wvec_{br}_{c}") for c in range(NH)]
                 for br in range(2)]
        for br in range(2):
            for c in range(NH):
                nc.sync.dma_start(sew1[br][c], sew1_d[br][c])
                nc.sync.dma_start(seb1[br][c], seb1_d[br][c])
                nc.sync.dma_start(sew2[br][c], sew2_d[br][c])
                nc.sync.dma_start(seb2[br][c], seb2_d[br][c])
                nc.sync.dma_start(wvec1[br][c], wv1_d[br][c])

        def taps_flat_tile(tbuf, r0):
            """9 contiguous slices (full padded rows) for padded-space conv
            over output padded rows r0+1..r0+TH (tile granularity, PE)."""
            base = 1 + (r0 + 1) * WP
            return [tbuf[:, base + dy * WP + dx: base + dy * WP + dx + TPAD]
                    for (dy, dx) in TAPS]

        def tap_bounds(j):
            """Whole-chunk padded-flat bounds for tap j: covers padded rows
            1..H (all interior rows; top/bottom pad rows excluded so reads
            stay within the slop), start/count adjusted to even element
            parity.  Only pad cells are dropped by the adjustments."""
            dy, dx = TAPS[j]
            delta = dy * WP + dx
            lo, cnt = WP, H * WP
            if (1 + lo + delta) % 2:
                lo, cnt = lo + 1, cnt - 1
            if cnt % 2:
                cnt -= 1
            return lo, cnt, delta

        def emit_body(rep):
            sfx = f"_r{rep}" if cfg['repeat'] > 1 else ""
            s_scale = [[[None] * NH for _ in range(2)] for _ in range(BL)]

            def phase1a(b):
                # ============ PHASE 1a: q and k GEMMs ============
                for br in range(2):
                    w_d = wq_d if br == 0 else wk_d
                    p3 = qpad3 if br == 0 else kpad3
                    w_sb = []
                    for kc in range(NH):
                        row = []
                        for oc in range(NH):
                            wt = wpool.tile([HD, HD], gdt, tag=f"wA{kc}_{oc}",
                                            name=f"wA{kc}_{oc}_b{b}_{br}{sfx}")
                            nc.sync.dma_start(wt, w_d[kc * HD:(kc + 1) * HD,
                                                      oc * HD:(oc + 1) * HD])
                            row.append(wt)
                        w_sb.append(row)
                    for t in range(NT):
                        r0 = t * TH
                        xt = []
                        for kc in range(NH):
                            xx = xpool.tile([HD, TN], gdt, tag=f"xt{kc}",
                                            name=f"xt{kc}_b{b}_{br}_{t}{sfx}")
                            nc.sync.dma_start(
                                xx.rearrange("p (h w) -> p h w", w=W),
                                x_d[b, kc * HD:(kc + 1) * HD, r0:r0 + TH, :])
                            xt.append(xx)
                        for oc in range(NH):
                            ps = mmpool.tile([HD, TN], f32, tag="mm",
                                             name=f"g{b}_{br}_{t}_{oc}{sfx}")
                            for kc in range(NH):
                                nc.tensor.matmul(ps, w_sb[kc][oc], xt[kc],
                                                 start=(kc == 0),
                                                 stop=(kc == NH - 1))
                            nc.scalar.copy(
                                p3[oc][:, 1 + r0:1 + r0 + TH, 1:1 + W],
                                ps.rearrange("p (h w) -> p h w", w=W))

            def phase1b(b, br):
                # ============ PHASE 1b: conv1 + SE for one branch ============
                if True:
                    srcb = qpad if br == 0 else kpad
                    src3 = qpad3 if br == 0 else kpad3
                    dstb = dwq if br == 0 else dwk
                    dst3 = dwq3 if br == 0 else dwk3
                    for oc in range(NH):
                        eng = cfg['conv1_assign'][(br, oc)]
                        stats = statpool.tile([HD, NT], f32, tag="stats",
                                              name=f"st{b}_{br}_{oc}{sfx}")
                        if eng == 'pe':
                            dgs = []
                            for j in range(9):
                                dg = dgpool.tile([HD, HD], cdt, tag=f"dg{j}",
                                                 name=f"dg{j}_{b}_{br}_{oc}{sfx}")
                                nc.sync.dma_start(dg, dg1_d[br][oc, j])
                                dgs.append(dg)
                            for t in range(NT):
                                r0 = t * TH
                                ps = mmpool.tile([HD, TPAD], f32, tag="mm",
                                                 name=f"c1{b}_{br}_{t}_{oc}{sfx}")
                                for j, v in enumerate(taps_flat_tile(srcb[oc], r0)):
                                    nc.tensor.matmul(ps, dgs[j], v,
                                                     start=(j == 0), stop=(j == 8))
                                nc.scalar.activation(
                                    dst3[oc][:, 1 + r0:1 + r0 + TH, 1:1 + W],
                                    ps.rearrange("p (h w) -> p h w",
                                                 w=WP)[:, :, 1:1 + W],
                                    AF.Identity, bias=bias1[br][oc],
                                    accum_out=stats[:, t:t + 1])
                            pooled_w = NT
                        else:
                            # hybrid DVE conv: even-parity taps as 4x
                            # tensor_scalar scaled copies, odd taps as ACT
                            # scaled copies ('dva') or 1x STT ('dve'); all
                            # accumulated by 2x tensor_tensor over a fixed
                            # 4-byte-aligned canonical range.
                            wvec = wvec1[br][oc]
                            clo, ccnt = WP + 1, H * WP - 2
                            order = dve_tap_order
                            j0 = order[0]            # an even tap
                            lo, cnt, delta = tap_bounds(j0)
                            cur, nxt, tmp = 0, 1, 2   # acc_g roles
                            nc.vector.tensor_scalar(
                                acc_g[cur][:, 1 + lo:1 + lo + cnt],
                                srcb[oc][:, 1 + lo + delta:1 + lo + delta + cnt],
                                wvec[:, j0:j0 + 1], bias1[br][oc],
                                AL.mult, AL.add)
                            for jj in order[1:8]:
                                lo, cnt, delta = tap_bounds(jj)
                                even = (delta % 2 == 0)
                                if even or eng == 'dva':
                                    src_sl = srcb[oc][:, 1 + lo + delta:
                                                      1 + lo + delta + cnt]
                                    t_sl = acc_g[tmp][:, 1 + lo:1 + lo + cnt]
                                    if even:
                                        nc.vector.tensor_scalar(
                                            t_sl, src_sl, wvec[:, jj:jj + 1],
                                            None, AL.mult)
                                    else:
                                        nc.scalar.activation(
                                            t_sl, src_sl, AF.Copy,
                                            scale=wvec[:, jj:jj + 1])
                                    nc.vector.tensor_tensor(
                                        acc_g[nxt][:, 1 + clo:1 + clo + ccnt],
                                        acc_g[cur][:, 1 + clo:1 + clo + ccnt],
                                        acc_g[tmp][:, 1 + clo:1 + clo + ccnt],
                                        AL.add)
                                    cur, nxt, tmp = nxt, tmp, cur
                                else:
                                    nc.vector.scalar_tensor_tensor(
                                        acc_g[nxt][:, 1 + lo:1 + lo + cnt],
                                        srcb[oc][:, 1 + lo + delta:
                                                 1 + lo + delta + cnt],
                                        wvec[:, jj:jj + 1],
                                        acc_g[cur][:, 1 + lo:1 + lo + cnt],
                                        AL.mult, AL.add)
                                    cur, nxt = nxt, cur
                            # last tap (odd parity): strided interior finalize
                            j8 = order[8]
                            dy, dx = TAPS[j8]
                            if eng == 'dva':
                                # ACT scaled copy of the last tap, 2x TT
                                # accumulate, ACT strided store + pooling
                                lo, cnt, delta = tap_bounds(j8)
                                t_sl = acc_g[tmp][:, 1 + lo:1 + lo + cnt]
                                nc.scalar.activation(
                                    t_sl,
                                    srcb[oc][:, 1 + lo + delta:
                                             1 + lo + delta + cnt],
                                    AF.Copy, scale=wvec[:, j8:j8 + 1])
                                nc.vector.tensor_tensor(
                                    acc_g[nxt][:, 1 + clo:1 + clo + ccnt],
                                    acc_g[cur][:, 1 + clo:1 + clo + ccnt],
                                    acc_g[tmp][:, 1 + clo:1 + clo + ccnt],
                                    AL.add)
                                nc.scalar.activation(
                                    dst3[oc][:, 1:1 + H, 1:1 + W],
                                    pad3(acc_g[nxt])[:, 1:1 + H, 1:1 + W],
                                    AF.Identity, bias=0.0,
                                    accum_out=stats[:, 0:1])
                            else:
                                nc.vector.scalar_tensor_tensor(
                                    dst3[oc][:, 1:1 + H, 1:1 + W],
                                    src3[oc][:, 1 + dy:1 + dy + H,
                                             1 + dx:1 + dx + W],
                                    wvec[:, j8:j8 + 1],
                                    pad3(acc_g[cur])[:, 1:1 + H, 1:1 + W],
                                    AL.mult, AL.add,
                                    accum_out=stats[:, 0:1])
                            pooled_w = 1
                        # ---- SE for this head ----
                        pooled = const.tile([HD, 1], f32, tag="pooled",
                                            bufs=4, name=f"pool{b}_{br}_{oc}{sfx}")
                        nc.vector.tensor_reduce(pooled, stats[:, 0:pooled_w],
                                                mybir.AxisListType.X, AL.add)
                        ps1 = sepool.tile([HD4, 1], f32, tag="se",
                                          name=f"se1_{b}_{br}_{oc}{sfx}")
                        nc.tensor.matmul(ps1, sew1[br][oc], pooled,
                                         start=True, stop=True)
                        hvec = const.tile([HD4, 1], f32, tag="hvec", bufs=4,
                                          name=f"h{b}_{br}_{oc}{sfx}")
                        nc.scalar.activation(hvec, ps1, AF.Relu,
                                             bias=seb1[br][oc])
                        ps2 = sepool.tile([HD, 1], f32, tag="se",
                                          name=f"se2_{b}_{br}_{oc}{sfx}")
                        nc.tensor.matmul(ps2, sew2[br][oc], hvec,
                                         start=True, stop=True)
                        s_sb = const.tile([HD, 1], f32, tag="s_scale", bufs=16,
                                          name=f"s{b}_{br}_{oc}{sfx}")
                        nc.scalar.activation(s_sb, ps2, AF.Sigmoid,
                                             bias=seb2[br][oc])
                        s_scale[b][br][oc] = s_sb

            def phase15(b):
                # ======== PHASE 1.5: m = s_q*dwq + s_k*dwk (into dwk) ======
                for oc in range(NH):
                    scr = acc_g[oc % 2]
                    nc.vector.tensor_scalar(scr, dwk[oc], s_scale[b][1][oc],
                                            None, AL.mult)
                    nc.vector.scalar_tensor_tensor(dwk[oc], dwq[oc],
                                                   s_scale[b][0][oc], scr,
                                                   AL.mult, AL.add)

            def phase2(b):
                # ================= PHASE 2 =================
                wv_sb, wp_sb = [], []
                for kc in range(NH):
                    rowv, rowp = [], []
                    for oc in range(NH):
                        wt = wpool.tile([HD, HD], gdt, tag=f"wV{kc}_{oc}",
                                        name=f"wV{kc}_{oc}_b{b}{sfx}")
                        nc.sync.dma_start(wt, wv_d[kc * HD:(kc + 1) * HD,
                                                   oc * HD:(oc + 1) * HD])
                        rowv.append(wt)
                        wt2 = wpool.tile([HD, HD], gdt, tag=f"wB{kc}_{oc}",
                                         name=f"wP{kc}_{oc}_b{b}{sfx}")
                        nc.sync.dma_start(wt2, wp_d[kc * HD:(kc + 1) * HD,
                                                    oc * HD:(oc + 1) * HD])
                        rowp.append(wt2)
                    wv_sb.append(rowv)
                    wp_sb.append(rowp)
                dg2 = []
                for oc in range(NH):
                    row = []
                    for j in range(9):
                        dg = dgpool.tile([HD, HD], cdt, tag=f"dg2_{oc}_{j}",
                                         name=f"dg2_{oc}_{j}_b{b}{sfx}")
                        nc.sync.dma_start(dg, dg2_d[oc, j])
                        row.append(dg)
                    dg2.append(row)
                for t in range(NT):
                    r0 = t * TH
                    xt = []
                    for kc in range(NH):
                        xx = xpool.tile([HD, TN], gdt, tag=f"xt{kc}",
                                        name=f"x2_{kc}_b{b}_{t}{sfx}")
                        nc.sync.dma_start(
                            xx.rearrange("p (h w) -> p h w", w=W),
                            x_d[b, kc * HD:(kc + 1) * HD, r0:r0 + TH, :])
                        xt.append(xx)
                    v_sb = []
                    for oc in range(NH):
                        ps = mmpool.tile([HD, TN], f32, tag="mm",
                                         name=f"v{b}_{t}_{oc}{sfx}")
                        for kc in range(NH):
                            nc.tensor.matmul(ps, wv_sb[kc][oc], xt[kc],
                                             start=(kc == 0), stop=(kc == NH - 1))
                        vv = vpool.tile([HD, TN], gdt, tag=f"vt{oc}",
                                        name=f"vt{oc}_b{b}_{t}{sfx}")
                        nc.scalar.copy(vv, ps)
                        v_sb.append(vv)
                    o2 = []
                    for oc in range(NH):
                        ps = mmpool.tile([HD, TPAD], f32, tag="mm",
                                         name=f"c2{b}_{t}_{oc}{sfx}")
                        for j, v in enumerate(taps_flat_tile(dwk[oc], r0)):
                            nc.tensor.matmul(ps, dg2[oc][j], v,
                                             start=(j == 0), stop=(j == 8))
                        c2t = o2pool.tile([HD, TN], gdt, tag="c2t", bufs=3,
                                          name=f"c2t_{oc}_b{b}_{t}{sfx}")
                        nc.scalar.activation(
                            c2t.rearrange("p (h w) -> p h w", w=W),
                            ps.rearrange("p (h w) -> p h w", w=WP)[:, :, 1:1 + W],
                            AF.Identity, bias=dwcb[oc])
                        oo = o2pool.tile([HD, TN], gdt, tag=f"o2_{oc}",
                                         name=f"o2_{oc}_b{b}_{t}{sfx}")
                        nc.vector.tensor_mul(oo, c2t, v_sb[oc])
                        o2.append(oo)
                    for oc in range(NH):
                        ps = mmpool.tile([HD, TN], f32, tag="mm",
                                         name=f"p{b}_{t}_{oc}{sfx}")
                        for kc in range(NH):
                            nc.tensor.matmul(ps, wp_sb[kc][oc], o2[kc],
                                             start=(kc == 0), stop=(kc == NH - 1))
                        # residual: convert x tile to f32 on ACT, then
                        # (proj + proj_b) + x with all-f32 operands
                        xc = otpool.tile([HD, TN], f32, tag=f"xc{oc}",
                                         name=f"xc{oc}_b{b}_{t}{sfx}")
                        nc.scalar.copy(xc, xt[oc])
                        ot = otpool.tile([HD, TN], f32, tag=f"ot{oc}",
                                         name=f"ot{oc}_b{b}_{t}{sfx}")
                        nc.vector.scalar_tensor_tensor(ot, ps, projb[oc],
                                                       xc, AL.add, AL.add)
                        nc.sync.dma_start(
                            out_d[b, oc * HD:(oc + 1) * HD, r0:r0 + TH, :],
                            ot.rearrange("p (h w) -> p h w", w=W))

            if BL == 2 and cfg.get('interleave', False):
                # interleave samples so DVE conv work overlaps PE phases
                phase1a(0)
                phase1b(0, 0)
                phase1b(0, 1)
                phase1a(1)
                phase15(0)
                phase1b(1, 0)
                phase2(0)
                phase1b(1, 1)
                phase15(1)
                phase2(1)
            else:
                for b in range(BL):
                    phase1a(b)
                    phase1b(b, 0)
                    phase1b(b, 1)
                    phase15(b)
                    phase2(b)

        if cfg['repeat'] > 1:
            for rep in range(cfg['repeat']):
                emit_body(rep)
        else:
            emit_body(0)

    nc.compile()
    return nc


# ---------------------------------------------------------------------------
# host-side weight prep
# ---------------------------------------------------------------------------

def prep_weights(inputs, cfg):
    import ml_dtypes
    conv_np = ml_dtypes.bfloat16 if cfg['conv_bf16'] else np.float32
    f32 = np.float32
    bf = ml_dtypes.bfloat16 if cfg.get('gemm_bf16', True) else np.float32
    qkv_w = np.asarray(inputs['qkv_w'], f32)
    wq_t = np.ascontiguousarray(qkv_w[0:DIM].T).astype(bf)
    wk_t = np.ascontiguousarray(qkv_w[DIM:2 * DIM].T).astype(bf)
    wv_t = np.ascontiguousarray(qkv_w[2 * DIM:3 * DIM].T).astype(bf)
    proj_t = np.ascontiguousarray(np.asarray(inputs['proj_w'], f32).T).astype(bf)

    def diag_taps(wconv):
        w = np.asarray(wconv, f32).reshape(DIM, 9)
        out = np.zeros((NH, 9, HD, HD), f32)
        idx = np.arange(HD)
        for c in range(NH):
            for j in range(9):
                out[c, j, idx, idx] = w[c * HD:(c + 1) * HD, j]
        return out.astype(conv_np)

    def wvecs(wconv):
        w = np.asarray(wconv, f32).reshape(DIM, 9)
        return np.ascontiguousarray(w.reshape(NH, HD, 9))

    npix = cfg['H'] * W
    return dict(
        wq_t=wq_t, wk_t=wk_t, wv_t=wv_t, proj_t=proj_t,
        diag1q=diag_taps(inputs['sq_w']),
        diag1k=diag_taps(inputs['sk_w']),
        diag2=diag_taps(inputs['dwc_w']),
        wvec1q=wvecs(inputs['sq_w']),
        wvec1k=wvecs(inputs['sk_w']),
        sq_b=np.asarray(inputs['sq_b'], f32).reshape(DIM, 1),
        sk_b=np.asarray(inputs['sk_b'], f32).reshape(DIM, 1),
        dwc_b=np.asarray(inputs['dwc_b'], f32).reshape(DIM, 1),
        proj_b=np.asarray(inputs['proj_b'], f32).reshape(DIM, 1),
        se_w1q=np.ascontiguousarray(
            np.asarray(inputs['cq_w1'], f32).transpose(0, 2, 1) / npix),
        se_b1q=np.asarray(inputs['cq_b1'], f32).reshape(NH, HD4, 1),
        se_w2q=np.ascontiguousarray(
            np.asarray(inputs['cq_w2'], f32).transpose(0, 2, 1)),
        se_b2q=np.asarray(inputs['cq_b2'], f32).reshape(NH, HD, 1),
        se_w1k=np.ascontiguousarray(
            np.asarray(inputs['ck_w1'], f32).transpose(0, 2, 1) / npix),
        se_b1k=np.asarray(inputs['ck_b1'], f32).reshape(NH, HD4, 1),
        se_w2k=np.ascontiguousarray(
            np.asarray(inputs['ck_w2'], f32).transpose(0, 2, 1)),
        se_b2k=np.asarray(inputs['ck_b2'], f32).reshape(NH, HD, 1),
    )


_CACHE = {}


def _get_compiled(cfg_key, cfg):
    if cfg_key not in _CACHE:
        _CACHE[cfg_key] = build_nc(cfg)
    return _CACHE[cfg_key]


def kernel(**inputs):
    import ml_dtypes
    from concourse import bass_utils
    cfg = default_cfg()
    nc = _get_compiled('main', cfg)
    w = prep_weights(inputs, cfg)
    x32 = np.asarray(inputs['x'], np.float32)
    x = x32.astype(ml_dtypes.bfloat16) if cfg['gemm_bf16'] else x32
    BL = cfg['b_local']
    in_maps = []
    for core in range(N_CORES):
        m = dict(w)
        m['x'] = np.ascontiguousarray(x[core * BL:(core + 1) * BL])
        in_maps.append(m)
    res = bass_utils.run_bass_kernel_spmd(nc, in_maps, core_ids=list(range(N_CORES)))
    out = np.empty((B, DIM, H_FULL, W), np.float32)
    for core in range(N_CORES):
        out[core * BL:(core + 1) * BL] = res.results[core]['out']
    return out



# revision 9
# speedup vs baseline: 1.4594x; 1.4594x over previous
"""Trainium2 Bass kernel for nn_CASAtt_MultiHead_v1 (CAS attention block).

Reference computation (per sample):
    qkv = 1x1 conv (qkv_w) -> q, k, v                        [512, 56, 56] each
    q <- SE(dwconv3x3(q, sq_w, sq_b))   (per-head squeeze-excite)
    k <- SE(dwconv3x3(k, sk_w, sk_b))
    out = proj(dwconv3x3(q + k, dwc_w, dwc_b) * v) + proj_b + x

Distribution: data-parallel over batch, 2 samples per NeuronCore x 8 cores.

v2 design: everything streaming through the PE is fp8 (e4m3) using
MatmulPerfMode.DoubleRow, which packs 2 fp8 weight planes per PE cell and
streams 2 moving planes at once -- measured 1.79x over bf16 for the
conv tap groups and ~2x for the GEMMs.  Depthwise 3x3 convs run as
diagonal-matrix matmuls accumulated in PSUM; taps are PAIRED into
DoubleRow matmuls via strided 3D access patterns ([128, 2, FD] with an
arbitrary plane stride, verified on hw):
  - conv domain is zero-padded HPxWP with WP=64 so vertical tap pairs
    (dy=-1,+1) are plane-stride 2*WP=128 views of the same buffer;
  - conv1 on q and conv1 on k accumulate into the SAME PSUM tile
    (producing m = s_q*conv(q) + s_k*conv(k) directly); their middle-row
    taps pair cross-branch (plane stride BUFN between q and k planes);
  - conv2's middle-row horizontal pair uses plane stride 2.
The SE sigmoid scales s are computed BEFORE conv1 from border-corrected
whole-image sums (mean(conv(q)) == A*S - B.R/C strips + corner terms,
exact for 'SAME' zero padding), so s folds into conv1's stationary
diagonals (scaled on DVE), phase-1.5 of the old design disappears, and
conv1 runs once instead of twice+add.  Weights are scaled x32 host-side
(fp8 dynamic range) and unscaled for free in the ACT drains.  proj_b is
pre-added into the f32 residual copy of x host-side.
"""

import numpy as np

DIM = 512
NH = 4
HD = 128
HD4 = 32
B, H_FULL, W = 16, 56, 56
N_CORES = 8

H = 56
WP = 64
HP = 58
PADN = HP * WP          # 3712
HEAD = 16               # head slop so (row0-1, col-1) reads stay in-bounds
BUFN = HEAD + PADN + 16  # 3744, multiple of 16
TH = 8
NT = H // TH            # 7
TN = TH * W             # 448
FDC = TH * WP           # 512  (conv matmul moving/psum free size)

SCALE_W = 32.0          # host-side scale on qkv/conv1/conv2/proj weights
CSCALE = 16.0           # extra scale kept on c2t/o2 for fp8 range


def default_cfg():
    return dict(
        b_local=B // N_CORES,
        conv2_midpair=True,
    )


def build_nc(cfg):
    import concourse.bass as bass
    import concourse.mybir as mybir
    import concourse.tile as tile
    from concourse import bacc
    from contextlib import ExitStack

    f32 = mybir.dt.float32
    f8 = mybir.dt.float8e4
    DR = mybir.MatmulPerfMode.DoubleRow
    AF = mybir.ActivationFunctionType
    AL = mybir.AluOpType
    AX = mybir.AxisListType

    BL = cfg['b_local']

    nc = bacc.Bacc("TRN2", target_bir_lowering=False, debug=False,
                   enable_asserts=False, num_devices=N_CORES)

    # ---------------- DRAM I/O ----------------
    x8_d = nc.dram_tensor("x8", [BL, DIM, H, W], f8, kind="ExternalInput").ap()
    xf_d = nc.dram_tensor("xf", [BL, DIM, H, W], f32, kind="ExternalInput").ap()
    out_d = nc.dram_tensor("out", [BL, DIM, H, W], f32, kind="ExternalOutput").ap()
    wg_d = {n: nc.dram_tensor(n, [HD, NH, DIM], f8, kind="ExternalInput").ap()
            for n in ("wq", "wk", "wv", "wp")}
    dg1_d = nc.dram_tensor("dg1", [NH, 9, HD, 2 * HD], f8,
                           kind="ExternalInput").ap()
    dg2p_d = nc.dram_tensor("dg2p", [NH, 4, HD, 2 * HD], f8,
                            kind="ExternalInput").ap()
    dg2s_d = nc.dram_tensor("dg2s", [NH, HD, HD], f8, kind="ExternalInput").ap()
    pv_d = nc.dram_tensor("pv", [2, DIM, 9], f32, kind="ExternalInput").ap()
    sew1_d = nc.dram_tensor("sew1", [2, NH, HD, HD4], f32,
                            kind="ExternalInput").ap()
    seb1_d = nc.dram_tensor("seb1", [2, NH, HD4, 1], f32,
                            kind="ExternalInput").ap()
    sew2_d = nc.dram_tensor("sew2", [2, NH, HD4, HD], f32,
                            kind="ExternalInput").ap()
    seb2_d = nc.dram_tensor("seb2", [2, NH, HD, 1], f32,
                            kind="ExternalInput").ap()
    b1_d = nc.dram_tensor("b1", [2, DIM, 1], f32, kind="ExternalInput").ap()
    dwcb_d = nc.dram_tensor("dwcb", [DIM, 1], f32, kind="ExternalInput").ap()

    def pairify(ap2d, start, fd, pstride):
        """[128, fd] slice at `start` -> [128, 2, fd] with plane stride."""
        u = ap2d[:, start:start + fd].unsqueeze(1)
        a = u.ap
        a.pop(1)
        a.insert(1, (pstride, 2))
        return u

    with tile.TileContext(nc) as tc, ExitStack() as ctx:
        const = ctx.enter_context(tc.tile_pool(name="const", bufs=1))
        small = ctx.enter_context(tc.tile_pool(name="small", bufs=24))
        wres = ctx.enter_context(tc.tile_pool(name="wres", bufs=1))
        dg1s_p = ctx.enter_context(tc.tile_pool(name="dg1s", bufs=2))
        big = ctx.enter_context(tc.tile_pool(name="big", bufs=1))
        x8pool = ctx.enter_context(tc.tile_pool(name="x8p", bufs=2))
        xfpool = ctx.enter_context(tc.tile_pool(name="xfp", bufs=2))
        c2pool = ctx.enter_context(tc.tile_pool(name="c2p", bufs=3))
        stage = ctx.enter_context(tc.tile_pool(name="stage", bufs=4))
        statp = ctx.enter_context(tc.tile_pool(name="statp", bufs=4))
        mmp = ctx.enter_context(tc.tile_pool(name="mmp", bufs=5, space="PSUM"))
        sep = ctx.enter_context(tc.tile_pool(name="sep", bufs=2, space="PSUM"))

        # ---------- persistent SBUF ----------
        # flat conv-domain buffers: qk[oc] holds q plane at [HEAD, HEAD+PADN)
        # and k plane at [BUFN+HEAD, ...); m[oc] single plane.
        qk = [big.tile([HD, 2 * BUFN], f8, name=f"qk{c}") for c in range(NH)]
        m_t = [big.tile([HD, BUFN], f8, name=f"m{c}") for c in range(NH)]
        v_t = big.tile([HD, NH, H * W], f8, name="v")
        o2_t = big.tile([HD, NH, H * W], f8, name="o2")

        # zero only the pad cells that valid conv outputs read:
        # row 0, row 57 (cols 0..57), col 0, col 57 (rows 0..57)
        def zero_pads(plane_base, tbuf):
            for start, stride, count in (
                    (plane_base + HEAD, 1, 58),
                    (plane_base + HEAD + 57 * WP, 1, 58),
                    (plane_base + HEAD, WP, 58),
                    (plane_base + HEAD + 57, WP, 58)):
                u = tbuf[:, start:start + 1 + stride * (count - 1)]
                a = u.ap
                a.pop(1)
                a.insert(1, (stride, count))
                nc.vector.memset(u, 0.0)

        for c in range(NH):
            zero_pads(0, qk[c])
            zero_pads(BUFN, qk[c])
            zero_pads(0, m_t[c])

        # ---------- resident weights ----------
        wg = {}
        for n in ("wq", "wk", "wv", "wp"):
            t = wres.tile([HD, NH, DIM], f8, name=f"{n}_sb")
            nc.sync.dma_start(t, wg_d[n])
            wg[n] = t
        dg1 = []
        for c in range(NH):
            t = wres.tile([HD, 9, 2 * HD], f8, name=f"dg1_{c}")
            nc.sync.dma_start(t, dg1_d[c].rearrange("j p f -> p j f"))
            dg1.append(t)
        dg2p = []
        for c in range(NH):
            t = wres.tile([HD, 4, 2 * HD], f8, name=f"dg2p_{c}")
            nc.sync.dma_start(t, dg2p_d[c].rearrange("j p f -> p j f"))
            dg2p.append(t)
        dg2s = []
        for c in range(NH):
            t = wres.tile([HD, HD], f8, name=f"dg2s_{c}")
            nc.sync.dma_start(t, dg2s_d[c])
            dg2s.append(t)
        pv = [[None] * NH for _ in range(2)]
        sew1 = [[None] * NH for _ in range(2)]
        seb1 = [[None] * NH for _ in range(2)]
        sew2 = [[None] * NH for _ in range(2)]
        seb2 = [[None] * NH for _ in range(2)]
        bias1 = [[None] * NH for _ in range(2)]
        dwcb = [None] * NH
        for br in range(2):
            for c in range(NH):
                sl = slice(c * HD, (c + 1) * HD)
                pv[br][c] = const.tile([HD, 9], f32, name=f"pv{br}_{c}")
                nc.sync.dma_start(pv[br][c], pv_d[br, sl])
                sew1[br][c] = const.tile([HD, HD4], f32, name=f"sw1{br}_{c}")
                nc.sync.dma_start(sew1[br][c], sew1_d[br, c])
                seb1[br][c] = const.tile([HD4, 1], f32, name=f"sb1{br}_{c}")
                nc.sync.dma_start(seb1[br][c], seb1_d[br, c])
                sew2[br][c] = const.tile([HD4, HD], f32, name=f"sw2{br}_{c}")
                nc.sync.dma_start(sew2[br][c], sew2_d[br, c])
                seb2[br][c] = const.tile([HD, 1], f32, name=f"sb2{br}_{c}")
                nc.sync.dma_start(seb2[br][c], seb2_d[br, c])
                bias1[br][c] = const.tile([HD, 1], f32, name=f"b1{br}_{c}")
                nc.sync.dma_start(bias1[br][c], b1_d[br, sl])
        for c in range(NH):
            dwcb[c] = const.tile([HD, 1], f32, name=f"dwcb{c}")
            nc.sync.dma_start(dwcb[c], dwcb_d[c * HD:(c + 1) * HD])

        def interior(tbuf, plane_base, t):
            """[128, 8, 56] view of padded rows 1+8t..8+8t, cols 1..56."""
            base = plane_base + HEAD + (1 + TH * t) * WP + 1
            u = tbuf[:, base:base + TH * WP]
            return u.rearrange("p (r c) -> p r c", c=WP)[:, :, 0:W]

        def emit_sample(b):
            sfx = f"_b{b}"
            # ---------------- qk GEMM ----------------
            x8 = x8pool.tile([HD, NH, H * W], f8, tag="x8", name=f"x8{sfx}")
            nc.sync.dma_start(
                x8, x8_d[b].rearrange("(kc p) h w -> p kc (h w)", p=HD))
            stats = [[None] * NH for _ in range(2)]
            for br in range(2):
                wt = wg["wq"] if br == 0 else wg["wk"]
                for oc in range(NH):
                    st = statp.tile([HD, NT], f32, tag="st",
                                    name=f"st{br}_{oc}{sfx}")
                    stats[br][oc] = st
                    for t in range(NT):
                        ps = mmp.tile([HD, TN], f32, tag="mm",
                                      name=f"g{br}_{oc}_{t}{sfx}")
                        for kp in range(2):
                            nc.tensor.matmul(
                                ps,
                                wt[:, 2 * kp:2 * kp + 2,
                                   oc * HD:(oc + 1) * HD],
                                x8[:, 2 * kp:2 * kp + 2,
                                   t * TN:(t + 1) * TN],
                                start=(kp == 0), stop=(kp == 1),
                                perf_mode=DR)
                        nc.scalar.activation(
                            interior(qk[oc], br * BUFN, t),
                            ps.rearrange("p (r c) -> p r c", c=W),
                            AF.Identity, bias=0.0, scale=1.0 / SCALE_W,
                            accum_out=st[:, t:t + 1])

            # ---------------- v GEMM ----------------
            for oc in range(NH):
                for t in range(NT):
                    ps = mmp.tile([HD, TN], f32, tag="mm",
                                  name=f"v{oc}_{t}{sfx}")
                    for kp in range(2):
                        nc.tensor.matmul(
                            ps,
                            wg["wv"][:, 2 * kp:2 * kp + 2,
                                     oc * HD:(oc + 1) * HD],
                            x8[:, 2 * kp:2 * kp + 2, t * TN:(t + 1) * TN],
                            start=(kp == 0), stop=(kp == 1), perf_mode=DR)
                    nc.scalar.activation(
                        v_t[:, oc, t * TN:(t + 1) * TN], ps,
                        AF.Identity, bias=0.0, scale=1.0 / SCALE_W)

            # ------------- pooled stats + SE -> s -------------
            s_sb = [[None] * NH for _ in range(2)]
            for br in range(2):
                for oc in range(NH):
                    pb = br * BUFN
                    # whole-image sum from drain accumulators
                    ssum = small.tile([HD, 1], f32, tag="ssum",
                                      name=f"ss{br}_{oc}{sfx}")
                    nc.vector.tensor_reduce(ssum, stats[br][oc][:, 0:NT],
                                            AX.X, AL.add)
                    strips = []
                    for start, stride in (
                            (pb + HEAD + WP + 1, 1),          # row 1
                            (pb + HEAD + 56 * WP + 1, 1),     # row 56
                            (pb + HEAD + WP + 1, WP),         # col 1
                            (pb + HEAD + WP + 56, WP)):       # col 56
                        u = qk[oc][:, start:start + 1 + stride * (W - 1)]
                        a = u.ap
                        a.pop(1)
                        a.insert(1, (stride, W))
                        rs = small.tile([HD, 1], f32, tag="rs", name=None)
                        nc.vector.tensor_reduce(rs, u, AX.X, AL.add)
                        strips.append(rs)
                    p_v = pv[br][oc]
                    acc = small.tile([HD, 1], f32, tag="acc", name=None)
                    nc.vector.tensor_scalar(acc, ssum, p_v[:, 0:1], None,
                                            AL.mult)
                    acc2 = small.tile([HD, 1], f32, tag="acc", name=None)
                    ops = [(strips[0], 1), (strips[1], 2), (strips[2], 3),
                           (strips[3], 4)]
                    corners = ((WP + 1, 5), (WP + 56, 6),
                               (56 * WP + 1, 7), (56 * WP + 56, 8))
                    cur = acc
                    for src, j in ops:
                        nxt = small.tile([HD, 1], f32, tag="acc", name=None)
                        nc.vector.scalar_tensor_tensor(
                            nxt, src, p_v[:, j:j + 1], cur, AL.mult, AL.add)
                        cur = nxt
                    for off, j in corners:
                        nxt = small.tile([HD, 1], f32, tag="acc", name=None)
                        nc.vector.scalar_tensor_tensor(
                            nxt, qk[oc][:, pb + HEAD + off:pb + HEAD + off + 1],
                            p_v[:, j:j + 1], cur, AL.mult, AL.add)
                        cur = nxt
                    # SE
                    ps1 = sep.tile([HD4, 1], f32, tag="se",
                                   name=f"se1_{br}_{oc}{sfx}")
                    nc.tensor.matmul(ps1, sew1[br][oc], cur,
                                     start=True, stop=True)
                    hv = small.tile([HD4, 1], f32, tag="hv", name=None)
                    nc.scalar.activation(hv, ps1, AF.Relu, bias=seb1[br][oc])
                    ps2 = sep.tile([HD, 1], f32, tag="se",
                                   name=f"se2_{br}_{oc}{sfx}")
                    nc.tensor.matmul(ps2, sew2[br][oc], hv,
                                     start=True, stop=True)
                    sv = small.tile([HD, 1], f32, tag="sv",
                                    name=f"s{br}_{oc}{sfx}")
                    nc.scalar.activation(sv, ps2, AF.Sigmoid,
                                         bias=seb2[br][oc])
                    s_sb[br][oc] = sv

            # ------- scale conv1 stationaries + bias_m -------
            dg1s = []
            bias_m = []
            for oc in range(NH):
                gt = dg1s_p.tile([HD, 9, 2 * HD], f8, tag=f"dg1s{oc}",
                                 name=f"dg1s{oc}{sfx}")
                for i in range(9):
                    if i < 3:
                        nc.vector.tensor_scalar(gt[:, i, :], dg1[oc][:, i, :],
                                                s_sb[0][oc], None, AL.mult)
                    elif i < 6:
                        nc.vector.tensor_scalar(gt[:, i, :], dg1[oc][:, i, :],
                                                s_sb[1][oc], None, AL.mult)
                    else:
                        nc.vector.tensor_scalar(
                            gt[:, i, 0:HD], dg1[oc][:, i, 0:HD],
                            s_sb[0][oc], None, AL.mult)
                        nc.vector.tensor_scalar(
                            gt[:, i, HD:2 * HD], dg1[oc][:, i, HD:2 * HD],
                            s_sb[1][oc], None, AL.mult)
                dg1s.append(gt)
                bm = small.tile([HD, 1], f32, tag="bm", name=f"bm{oc}{sfx}")
                tmp = small.tile([HD, 1], f32, tag="bmt", name=None)
                nc.vector.tensor_scalar(tmp, bias1[0][oc], s_sb[0][oc],
                                        None, AL.mult)
                nc.vector.scalar_tensor_tensor(bm, bias1[1][oc], s_sb[1][oc],
                                               tmp, AL.mult, AL.add)
                bias_m.append(bm)

            # ---------------- conv1 (fused q+k -> m) ----------------
            for oc in range(NH):
                qplane = qk[oc]
                for t in range(NT):
                    y0 = 1 + TH * t
                    ps = mmp.tile([HD, FDC], f32, tag="mm",
                                  name=f"c1_{oc}_{t}{sfx}")
                    mms = []
                    for i in range(9):
                        grp, dx = divmod(i, 3)
                        dx -= 1
                        if grp == 0:    # q vertical pairs
                            mv = pairify(qplane,
                                         HEAD + (y0 - 1) * WP + dx,
                                         FDC, 2 * WP)
                        elif grp == 1:  # k vertical pairs
                            mv = pairify(qplane,
                                         BUFN + HEAD + (y0 - 1) * WP + dx,
                                         FDC, 2 * WP)
                        else:           # cross q/k middle row
                            mv = pairify(qplane, HEAD + y0 * WP + dx,
                                         FDC, BUFN)
                        st = dg1s[oc][:, i, :]
                        st = st.rearrange("p (two f) -> p two f", two=2)
                        mms.append((st, mv))
                    for i, (st, mv) in enumerate(mms):
                        nc.tensor.matmul(ps, st, mv, start=(i == 0),
                                         stop=(i == 8), perf_mode=DR)
                    nc.scalar.activation(
                        interior(m_t[oc], 0, t),
                        ps.rearrange("p (r c) -> p r c", c=WP)[:, :, 1:1 + W],
                        AF.Identity, bias=bias_m[oc], scale=1.0 / SCALE_W)

            # ---------------- conv2 + o2 ----------------
            for oc in range(NH):
                for t in range(NT):
                    y0 = 1 + TH * t
                    ps = mmp.tile([HD, FDC], f32, tag="mm",
                                  name=f"c2_{oc}_{t}{sfx}")
                    for i in range(3):
                        dx = i - 1
                        st = dg2p[oc][:, i, :]
                        st = st.rearrange("p (two f) -> p two f", two=2)
                        mv = pairify(m_t[oc], HEAD + (y0 - 1) * WP + dx,
                                     FDC, 2 * WP)
                        nc.tensor.matmul(ps, st, mv, start=(i == 0),
                                         stop=False, perf_mode=DR)
                    if cfg['conv2_midpair']:
                        st = dg2p[oc][:, 3, :]
                        st = st.rearrange("p (two f) -> p two f", two=2)
                        mv = pairify(m_t[oc], HEAD + y0 * WP - 1, FDC, 2)
                        nc.tensor.matmul(ps, st, mv, start=False, stop=False,
                                         perf_mode=DR)
                    else:
                        for pl, dx in ((0, -1), (1, 1)):
                            st = dg2p[oc][:, 3, pl * HD:(pl + 1) * HD]
                            nc.tensor.matmul(
                                ps, st,
                                m_t[oc][:, HEAD + y0 * WP + dx:
                                        HEAD + y0 * WP + dx + FDC],
                                start=False, stop=False)
                    nc.tensor.matmul(
                        ps, dg2s[oc],
                        m_t[oc][:, HEAD + y0 * WP:HEAD + y0 * WP + FDC],
                        start=False, stop=True)
                    c2t = c2pool.tile([HD, TN], f8, tag="c2t",
                                      name=f"c2t{oc}_{t}{sfx}")
                    nc.scalar.activation(
                        c2t.rearrange("p (r c) -> p r c", c=W),
                        ps.rearrange("p (r c) -> p r c", c=WP)[:, :, 1:1 + W],
                        AF.Identity, bias=dwcb[oc], scale=CSCALE / SCALE_W)
                    nc.vector.tensor_tensor(
                        o2_t[:, oc, t * TN:(t + 1) * TN], c2t,
                        v_t[:, oc, t * TN:(t + 1) * TN], AL.mult)

            # ---------------- proj + residual ----------------
            for ocp in range(NH):
                xf = xfpool.tile([HD, H * W], f32, tag="xf",
                                 name=f"xf{ocp}{sfx}")
                nc.sync.dma_start(
                    xf.rearrange("p (h w) -> p h w", w=W),
                    xf_d[b, ocp * HD:(ocp + 1) * HD])
                for t in range(NT):
                    ps = mmp.tile([HD, TN], f32, tag="mm",
                                  name=f"p{ocp}_{t}{sfx}")
                    for kp in range(2):
                        nc.tensor.matmul(
                            ps,
                            wg["wp"][:, 2 * kp:2 * kp + 2,
                                     ocp * HD:(ocp + 1) * HD],
                            o2_t[:, 2 * kp:2 * kp + 2, t * TN:(t + 1) * TN],
                            start=(kp == 0), stop=(kp == 1), perf_mode=DR)
                    stg = stage.tile([HD, TN], f32, tag="stg",
                                     name=f"stg{ocp}_{t}{sfx}")
                    nc.vector.scalar_tensor_tensor(
                        stg, ps, 1.0 / (SCALE_W * CSCALE),
                        xf[:, t * TN:(t + 1) * TN], AL.mult, AL.add)
                    nc.sync.dma_start(
                        out_d[b, ocp * HD:(ocp + 1) * HD,
                              TH * t:TH * (t + 1), :],
                        stg.rearrange("p (r c) -> p r c", c=W))

        for b in range(BL):
            emit_sample(b)

    nc.compile()
    return nc


# ---------------------------------------------------------------------------
# host-side weight prep
# ---------------------------------------------------------------------------

def prep_weights(inputs, cfg):
    import ml_dtypes
    f8 = ml_dtypes.float8_e4m3
    f32 = np.float32

    qkv_w = np.asarray(inputs['qkv_w'], f32)
    proj_w = np.asarray(inputs['proj_w'], f32)

    def gemm_tile(wmat):
        # [HD p, NH kc, DIM oc*128+col] = SCALE_W * W[ocol, kc*128+p]
        arr = (SCALE_W * wmat).reshape(DIM, NH, HD).transpose(2, 1, 0)
        return np.ascontiguousarray(arr).astype(f8)

    wq = gemm_tile(qkv_w[0:DIM])
    wk = gemm_tile(qkv_w[DIM:2 * DIM])
    wv = gemm_tile(qkv_w[2 * DIM:3 * DIM])
    wp = gemm_tile(proj_w)

    sq = np.asarray(inputs['sq_w'], f32).reshape(DIM, 3, 3) * SCALE_W
    sk = np.asarray(inputs['sk_w'], f32).reshape(DIM, 3, 3) * SCALE_W
    dw = np.asarray(inputs['dwc_w'], f32).reshape(DIM, 3, 3) * SCALE_W

    idx = np.arange(HD)

    def diag(vals):
        d = np.zeros((HD, HD), f32)
        d[idx, idx] = vals
        return d

    # conv1 pair stationaries: [NH, 9, HD, 2*HD]
    dg1 = np.zeros((NH, 9, HD, 2 * HD), f32)
    for c in range(NH):
        q = sq[c * HD:(c + 1) * HD]
        k = sk[c * HD:(c + 1) * HD]
        for i in range(3):      # q vertical pairs, dx = i-1
            dg1[c, i, :, 0:HD] = diag(q[:, 0, i])
            dg1[c, i, :, HD:] = diag(q[:, 2, i])
        for i in range(3):      # k vertical pairs
            dg1[c, 3 + i, :, 0:HD] = diag(k[:, 0, i])
            dg1[c, 3 + i, :, HD:] = diag(k[:, 2, i])
        for i in range(3):      # cross middle row
            dg1[c, 6 + i, :, 0:HD] = diag(q[:, 1, i])
            dg1[c, 6 + i, :, HD:] = diag(k[:, 1, i])

    dg2p = np.zeros((NH, 4, HD, 2 * HD), f32)
    dg2s = np.zeros((NH, HD, HD), f32)
    for c in range(NH):
        d = dw[c * HD:(c + 1) * HD]
        for i in range(3):      # vertical pairs
            dg2p[c, i, :, 0:HD] = diag(d[:, 0, i])
            dg2p[c, i, :, HD:] = diag(d[:, 2, i])
        dg2p[c, 3, :, 0:HD] = diag(d[:, 1, 0])   # (0,-1)
        dg2p[c, 3, :, HD:] = diag(d[:, 1, 2])    # (0,+1)
        dg2s[c] = diag(d[:, 1, 1])

    # pooled-correction vectors (natural scale, /npix folded in):
    # pooled = A*S + B1*R0 + B2*R1 + B3*C0 + B4*C1 + c00*q00 + ...
    npix = float(H * W)
    pvv = np.zeros((2, DIM, 9), f32)
    # tap (dy,dx) reads rows [dy, 56+dy) clipped: dy=+1 excludes image row 0,
    # dy=-1 excludes row 55; dx likewise for cols.  Corner = doubly excluded.
    for br, wc in enumerate((sq / SCALE_W, sk / SCALE_W)):
        pvv[br, :, 0] = wc.sum(axis=(1, 2))
        pvv[br, :, 1] = -wc[:, 2, :].sum(axis=1)   # row 0 strip (dy=+1 taps)
        pvv[br, :, 2] = -wc[:, 0, :].sum(axis=1)   # row 55 strip (dy=-1)
        pvv[br, :, 3] = -wc[:, :, 2].sum(axis=1)   # col 0 strip (dx=+1)
        pvv[br, :, 4] = -wc[:, :, 0].sum(axis=1)   # col 55 strip (dx=-1)
        pvv[br, :, 5] = wc[:, 2, 2]                # q[0,0]
        pvv[br, :, 6] = wc[:, 2, 0]                # q[0,55]
        pvv[br, :, 7] = wc[:, 0, 2]                # q[55,0]
        pvv[br, :, 8] = wc[:, 0, 0]                # q[55,55]
    pvv /= npix

    sew1 = np.stack([
        np.asarray(inputs['cq_w1'], f32).transpose(0, 2, 1),
        np.asarray(inputs['ck_w1'], f32).transpose(0, 2, 1)])
    seb1 = np.stack([np.asarray(inputs['cq_b1'], f32).reshape(NH, HD4, 1),
                     np.asarray(inputs['ck_b1'], f32).reshape(NH, HD4, 1)])
    sew2 = np.stack([
        np.asarray(inputs['cq_w2'], f32).transpose(0, 2, 1),
        np.asarray(inputs['ck_w2'], f32).transpose(0, 2, 1)])
    seb2 = np.stack([np.asarray(inputs['cq_b2'], f32).reshape(NH, HD, 1),
                     np.asarray(inputs['ck_b2'], f32).reshape(NH, HD, 1)])
    b1 = np.stack([np.asarray(inputs['sq_b'], f32).reshape(DIM, 1),
                   np.asarray(inputs['sk_b'], f32).reshape(DIM, 1)])

    return dict(
        wq=wq, wk=wk, wv=wv, wp=wp,
        dg1=dg1.astype(f8), dg2p=dg2p.astype(f8), dg2s=dg2s.astype(f8),
        pv=pvv, sew1=sew1, seb1=seb1, sew2=sew2, seb2=seb2, b1=b1,
        dwcb=CSCALE * np.asarray(inputs['dwc_b'], f32).reshape(DIM, 1),
    )


_CACHE = {}


def _get_compiled(cfg_key, cfg):
    if cfg_key not in _CACHE:
        _CACHE[cfg_key] = build_nc(cfg)
    return _CACHE[cfg_key]


def make_in_maps(inputs, cfg):
    import ml_dtypes
    w = prep_weights(inputs, cfg)
    x32 = np.asarray(inputs['x'], np.float32)
    x8 = np.clip(x32, -240, 240).astype(ml_dtypes.float8_e4m3)
    projb = np.asarray(inputs['proj_b'], np.float32)
    xf = x32 + projb[None, :, None, None]
    BL = cfg['b_local']
    in_maps = []
    for core in range(N_CORES):
        mm = dict(w)
        mm['x8'] = np.ascontiguousarray(x8[core * BL:(core + 1) * BL])
        mm['xf'] = np.ascontiguousarray(xf[core * BL:(core + 1) * BL])
        in_maps.append(mm)
    return in_maps


def kernel(**inputs):
    from concourse import bass_utils
    cfg = default_cfg()
    nc = _get_compiled('main', cfg)
    in_maps = make_in_maps(inputs, cfg)
    res = bass_utils.run_bass_kernel_spmd(nc, in_maps,
                                          core_ids=list(range(N_CORES)))
    out = np.empty((B, DIM, H_FULL, W), np.float32)
    for core in range(N_CORES):
        out[core * BL:(core + 1) * BL] = res.results[core]['out']
    return out


# revision 18
# speedup vs baseline: 1.6708x; 1.1449x over previous
"""Trainium2 Bass kernel for nn_CASAtt_MultiHead_v1 (CAS attention block).

Reference computation (per sample):
    qkv = 1x1 conv (qkv_w) -> q, k, v                        [512, 56, 56] each
    q <- SE(dwconv3x3(q, sq_w, sq_b))   (per-head squeeze-excite)
    k <- SE(dwconv3x3(k, sk_w, sk_b))
    out = proj(dwconv3x3(q + k, dwc_w, dwc_b) * v) + proj_b + x

Distribution: data-parallel over batch, 2 samples per NeuronCore x 8 cores.

v2 design: everything streaming through the PE is fp8 (e4m3) using
MatmulPerfMode.DoubleRow, which packs 2 fp8 weight planes per PE cell and
streams 2 moving planes at once -- measured 1.79x over bf16 for the
conv tap groups and ~2x for the GEMMs.  Depthwise 3x3 convs run as
diagonal-matrix matmuls accumulated in PSUM; taps are PAIRED into
DoubleRow matmuls via strided 3D access patterns ([128, 2, FD] with an
arbitrary plane stride, verified on hw):
  - conv domain is zero-padded HPxWP with WP=64 so vertical tap pairs
    (dy=-1,+1) are plane-stride 2*WP=128 views of the same buffer;
  - conv1 on q and conv1 on k accumulate into the SAME PSUM tile
    (producing m = s_q*conv(q) + s_k*conv(k) directly); their middle-row
    taps pair cross-branch (plane stride BUFN between q and k planes);
  - conv2's middle-row horizontal pair uses plane stride 2.
The SE sigmoid scales s are computed BEFORE conv1 from border-corrected
whole-image sums (mean(conv(q)) == A*S - B.R/C strips + corner terms,
exact for 'SAME' zero padding), so s folds into conv1's stationary
diagonals (scaled on DVE), phase-1.5 of the old design disappears, and
conv1 runs once instead of twice+add.  Weights are scaled x32 host-side
(fp8 dynamic range) and unscaled for free in the ACT drains.  proj_b is
pre-added into the f32 residual copy of x host-side.
"""

import numpy as np

DIM = 512
NH = 4
HD = 128
HD4 = 32
B, H_FULL, W = 16, 56, 56
N_CORES = 8

H = 56
WP = 64
HP = 58
PADN = HP * WP          # 3712
HEAD = 16               # head slop so (row0-1, col-1) reads stay in-bounds
BUFN = HEAD + PADN + 16  # 3744, multiple of 16
TH = 8
NT = H // TH            # 7
TN = TH * W             # 448
FDC = TH * WP           # 512  (conv matmul moving/psum free size)

SCALE_W = 32.0          # host-side scale on qkv/conv1/conv2/proj weights
CSCALE = 16.0           # extra scale kept on c2t/o2 for fp8 range


def default_cfg():
    return dict(
        b_local=B // N_CORES,
        conv2_midpair=True,
    )


def build_nc(cfg):
    import concourse.bass as bass
    import concourse.mybir as mybir
    import concourse.tile as tile
    from concourse import bacc
    from contextlib import ExitStack

    f32 = mybir.dt.float32
    f8 = mybir.dt.float8e4
    DR = mybir.MatmulPerfMode.DoubleRow
    AF = mybir.ActivationFunctionType
    AL = mybir.AluOpType
    AX = mybir.AxisListType

    BL = cfg['b_local']

    nc = bacc.Bacc("TRN2", target_bir_lowering=False, debug=False,
                   enable_asserts=False, num_devices=N_CORES)

    # ---------------- DRAM I/O ----------------
    x8_d = nc.dram_tensor("x8", [BL, DIM, H, W], f8, kind="ExternalInput").ap()
    xf_d = nc.dram_tensor("xf", [BL, DIM, H, W], f32, kind="ExternalInput").ap()
    out_d = nc.dram_tensor("out", [BL, DIM, H, W], f32, kind="ExternalOutput").ap()
    wg_d = {n: nc.dram_tensor(n, [HD, NH, DIM], f8, kind="ExternalInput").ap()
            for n in ("wq", "wk", "wv", "wp")}
    dg1_d = nc.dram_tensor("dg1", [NH, 9, HD, 2 * HD], f8,
                           kind="ExternalInput").ap()
    dg2p_d = nc.dram_tensor("dg2p", [NH, 4, HD, 2 * HD], f8,
                            kind="ExternalInput").ap()
    dg2s_d = nc.dram_tensor("dg2s", [NH, HD, HD], f8, kind="ExternalInput").ap()
    # packed per-(br,oc) consts, HD partitions:
    #   cols 0-8 pv, 9-40 sew1, 41 seb2, 42 bias1, 43 dwcb (br=0 only)
    cpack_d = nc.dram_tensor("cpack", [2, NH, HD, 44], f32,
                             kind="ExternalInput").ap()
    # packed HD4-partition consts: col 0 seb1, cols 1-128 sew2
    spack_d = nc.dram_tensor("spack", [2, NH, HD4, 129], f32,
                             kind="ExternalInput").ap()

    def pairify(ap2d, start, fd, pstride):
        """[128, fd] slice at `start` -> [128, 2, fd] with plane stride."""
        u = ap2d[:, start:start + fd].unsqueeze(1)
        a = u.ap
        a.pop(1)
        a.insert(1, (pstride, 2))
        return u

    with tile.TileContext(nc) as tc, ExitStack() as ctx:
        const = ctx.enter_context(tc.tile_pool(name="const", bufs=1))
        small = ctx.enter_context(tc.tile_pool(name="small", bufs=24))
        wres = ctx.enter_context(tc.tile_pool(name="wres", bufs=1))
        dg1s_p = ctx.enter_context(tc.tile_pool(name="dg1s", bufs=1))
        big = ctx.enter_context(tc.tile_pool(name="big", bufs=1))
        x8pool = ctx.enter_context(tc.tile_pool(name="x8p", bufs=2))
        xfpool = ctx.enter_context(tc.tile_pool(name="xfp", bufs=4))
        c2pool = ctx.enter_context(tc.tile_pool(name="c2p", bufs=3))
        stage = ctx.enter_context(tc.tile_pool(name="stage", bufs=4))
        statp = ctx.enter_context(tc.tile_pool(name="statp", bufs=4))
        mmp = ctx.enter_context(tc.tile_pool(name="mmp", bufs=5, space="PSUM"))
        sep = ctx.enter_context(tc.tile_pool(name="sep", bufs=2, space="PSUM"))

        # ---------- persistent SBUF ----------
        # flat conv-domain buffers: qk[oc] holds q plane at [HEAD, HEAD+PADN)
        # and k plane at [BUFN+HEAD, ...); m[oc] single plane.
        qk = [big.tile([HD, 2 * BUFN], f8, name=f"qk{c}") for c in range(NH)]
        m_t = [big.tile([HD, BUFN], f8, name=f"m{c}") for c in range(NH)]
        v_t = big.tile([HD, NH, H * W], f8, name="v")
        o2_t = big.tile([HD, NH, H * W], f8, name="o2")

        # zero only the pad cells that valid conv outputs read:
        # row 0, row 57 (cols 0..57), col 0, col 57 (rows 0..57)
        def zero_pads(plane_base, tbuf):
            for start, stride, count in (
                    (plane_base + HEAD, 1, 58),
                    (plane_base + HEAD + 57 * WP, 1, 58),
                    (plane_base + HEAD, WP, 58),
                    (plane_base + HEAD + 57, WP, 58)):
                u = tbuf[:, start:start + 1 + stride * (count - 1)]
                a = u.ap
                a.pop(1)
                a.insert(1, (stride, count))
                nc.vector.memset(u, 0.0)

        for c in range(NH):
            zero_pads(0, qk[c])
            zero_pads(BUFN, qk[c])
            zero_pads(0, m_t[c])

        # ---------- resident weights ----------
        # ordering matters: the first sample's qk GEMM needs only wq/wk/x8,
        # so issue those DMAs first and defer everything else behind them.
        wg = {}
        for n in ("wq", "wk"):
            t = wres.tile([HD, NH, DIM], f8, name=f"{n}_sb")
            nc.sync.dma_start(t, wg_d[n])
            wg[n] = t
        x8_first = x8pool.tile([HD, NH, H * W], f8, tag="x8", name="x8_b0")
        nc.sync.dma_start(
            x8_first, x8_d[0].rearrange("(kc p) h w -> p kc (h w)", p=HD))
        for n in ("wv", "wp"):
            t = wres.tile([HD, NH, DIM], f8, name=f"{n}_sb")
            nc.sync.dma_start(t, wg_d[n])
            wg[n] = t
        cpack = wres.tile([HD, 2, NH, 44], f32, name="cpack_sb")
        nc.sync.dma_start(cpack, cpack_d.rearrange("b n p c -> p b n c"))
        spack = wres.tile([HD4, 2, NH, 129], f32, name="spack_sb")
        nc.sync.dma_start(spack, spack_d.rearrange("b n p c -> p b n c"))
        dg1 = []
        for c in range(NH):
            t = wres.tile([HD, 9, 2 * HD], f8, name=f"dg1_{c}")
            nc.sync.dma_start(t, dg1_d[c].rearrange("j p f -> p j f"))
            dg1.append(t)
        dg2p = []
        for c in range(NH):
            t = wres.tile([HD, 4, 2 * HD], f8, name=f"dg2p_{c}")
            nc.sync.dma_start(t, dg2p_d[c].rearrange("j p f -> p j f"))
            dg2p.append(t)
        dg2s = []
        for c in range(NH):
            t = wres.tile([HD, HD], f8, name=f"dg2s_{c}")
            nc.sync.dma_start(t, dg2s_d[c])
            dg2s.append(t)
        pv = [[cpack[:, br, c, 0:9] for c in range(NH)] for br in range(2)]
        sew1 = [[cpack[:, br, c, 9:41] for c in range(NH)] for br in range(2)]
        seb2 = [[cpack[:, br, c, 41:42] for c in range(NH)] for br in range(2)]
        bias1 = [[cpack[:, br, c, 42:43] for c in range(NH)] for br in range(2)]
        dwcb = [cpack[:, 0, c, 43:44] for c in range(NH)]
        seb1 = [[spack[:, br, c, 0:1] for c in range(NH)] for br in range(2)]
        sew2 = [[spack[:, br, c, 1:129] for c in range(NH)] for br in range(2)]

        def interior(tbuf, plane_base, t):
            """[128, 8, 56] view of padded rows 1+8t..8+8t, cols 1..56."""
            base = plane_base + HEAD + (1 + TH * t) * WP + 1
            u = tbuf[:, base:base + TH * WP]
            return u.rearrange("p (r c) -> p r c", c=WP)[:, :, 0:W]

        def emit_sample(b):
            sfx = f"_b{b}"
            # ---------------- input prefetch ----------------
            if b == 0:
                x8 = x8_first
            else:
                x8 = x8pool.tile([HD, NH, H * W], f8, tag="x8",
                                 name=f"x8{sfx}")
                nc.sync.dma_start(
                    x8, x8_d[b].rearrange("(kc p) h w -> p kc (h w)", p=HD))
            xfs = []
            for ocp in range(NH):
                xf = xfpool.tile([HD, H * W], f32, tag="xf",
                                 name=f"xf{ocp}{sfx}")
                nc.sync.dma_start(
                    xf.rearrange("p (h w) -> p h w", w=W),
                    xf_d[b, ocp * HD:(ocp + 1) * HD])
                xfs.append(xf)
            # ---------------- qk GEMM ----------------
            stats = [[None] * NH for _ in range(2)]
            for br in range(2):
                wt = wg["wq"] if br == 0 else wg["wk"]
                for oc in range(NH):
                    st = statp.tile([HD, NT], f32, tag="st",
                                    name=f"st{br}_{oc}{sfx}")
                    stats[br][oc] = st
                    for t in range(NT):
                        ps = mmp.tile([HD, TN], f32, tag="mm",
                                      name=f"g{br}_{oc}_{t}{sfx}")
                        for kp in range(2):
                            nc.tensor.matmul(
                                ps,
                                wt[:, 2 * kp:2 * kp + 2,
                                   oc * HD:(oc + 1) * HD],
                                x8[:, 2 * kp:2 * kp + 2,
                                   t * TN:(t + 1) * TN],
                                start=(kp == 0), stop=(kp == 1),
                                perf_mode=DR)
                        nc.scalar.activation(
                            interior(qk[oc], br * BUFN, t),
                            ps.rearrange("p (r c) -> p r c", c=W),
                            AF.Identity, bias=0.0, scale=1.0 / SCALE_W,
                            accum_out=st[:, t:t + 1])

            # ---------------- v GEMM ----------------
            for oc in range(NH):
                for t in range(NT):
                    ps = mmp.tile([HD, TN], f32, tag="mm",
                                  name=f"v{oc}_{t}{sfx}")
                    for kp in range(2):
                        nc.tensor.matmul(
                            ps,
                            wg["wv"][:, 2 * kp:2 * kp + 2,
                                     oc * HD:(oc + 1) * HD],
                            x8[:, 2 * kp:2 * kp + 2, t * TN:(t + 1) * TN],
                            start=(kp == 0), stop=(kp == 1), perf_mode=DR)
                    nc.scalar.activation(
                        v_t[:, oc, t * TN:(t + 1) * TN], ps,
                        AF.Identity, bias=0.0, scale=1.0 / SCALE_W)

            # ------------- pooled stats + SE -> s -------------
            s_sb = [[None] * NH for _ in range(2)]
            for br in range(2):
                for oc in range(NH):
                    pb = br * BUFN
                    # whole-image sum from drain accumulators
                    ssum = small.tile([HD, 1], f32, tag="ssum",
                                      name=f"ss{br}_{oc}{sfx}")
                    nc.vector.tensor_reduce(ssum, stats[br][oc][:, 0:NT],
                                            AX.X, AL.add)
                    strips = []
                    for start, stride in (
                            (pb + HEAD + WP + 1, 1),          # row 1
                            (pb + HEAD + 56 * WP + 1, 1),     # row 56
                            (pb + HEAD + WP + 1, WP),         # col 1
                            (pb + HEAD + WP + 56, WP)):       # col 56
                        u = qk[oc][:, start:start + 1 + stride * (W - 1)]
                        a = u.ap
                        a.pop(1)
                        a.insert(1, (stride, W))
                        rs = small.tile([HD, 1], f32, tag="rs", name=None)
                        nc.vector.tensor_reduce(rs, u, AX.X, AL.add)
                        strips.append(rs)
                    p_v = pv[br][oc]
                    acc = small.tile([HD, 1], f32, tag="acc", name=None)
                    nc.vector.tensor_scalar(acc, ssum, p_v[:, 0:1], None,
                                            AL.mult)
                    acc2 = small.tile([HD, 1], f32, tag="acc", name=None)
                    ops = [(strips[0], 1), (strips[1], 2), (strips[2], 3),
                           (strips[3], 4)]
                    corners = ((WP + 1, 5), (WP + 56, 6),
                               (56 * WP + 1, 7), (56 * WP + 56, 8))
                    cur = acc
                    for src, j in ops:
                        nxt = small.tile([HD, 1], f32, tag="acc", name=None)
                        nc.vector.scalar_tensor_tensor(
                            nxt, src, p_v[:, j:j + 1], cur, AL.mult, AL.add)
                        cur = nxt
                    for off, j in corners:
                        nxt = small.tile([HD, 1], f32, tag="acc", name=None)
                        nc.vector.scalar_tensor_tensor(
                            nxt, qk[oc][:, pb + HEAD + off:pb + HEAD + off + 1],
                            p_v[:, j:j + 1], cur, AL.mult, AL.add)
                        cur = nxt
                    # SE
                    ps1 = sep.tile([HD4, 1], f32, tag="se",
                                   name=f"se1_{br}_{oc}{sfx}")
                    nc.tensor.matmul(ps1, sew1[br][oc], cur,
                                     start=True, stop=True)
                    hv = small.tile([HD4, 1], f32, tag="hv", name=None)
                    nc.scalar.activation(hv, ps1, AF.Relu, bias=seb1[br][oc])
                    ps2 = sep.tile([HD, 1], f32, tag="se",
                                   name=f"se2_{br}_{oc}{sfx}")
                    nc.tensor.matmul(ps2, sew2[br][oc], hv,
                                     start=True, stop=True)
                    sv = small.tile([HD, 1], f32, tag="sv",
                                    name=f"s{br}_{oc}{sfx}")
                    nc.scalar.activation(sv, ps2, AF.Sigmoid,
                                         bias=seb2[br][oc])
                    s_sb[br][oc] = sv

            # ------- scale conv1 stationaries + bias_m -------
            dg1s = []
            bias_m = []
            for oc in range(NH):
                gt = dg1s_p.tile([HD, 9, 2 * HD], f8, tag=f"dg1s{oc}",
                                 name=f"dg1s{oc}{sfx}")
                for i in range(9):
                    if i < 3:
                        nc.vector.tensor_scalar(gt[:, i, :], dg1[oc][:, i, :],
                                                s_sb[0][oc], None, AL.mult)
                    elif i < 6:
                        nc.vector.tensor_scalar(gt[:, i, :], dg1[oc][:, i, :],
                                                s_sb[1][oc], None, AL.mult)
                    else:
                        nc.vector.tensor_scalar(
                            gt[:, i, 0:HD], dg1[oc][:, i, 0:HD],
                            s_sb[0][oc], None, AL.mult)
                        nc.vector.tensor_scalar(
                            gt[:, i, HD:2 * HD], dg1[oc][:, i, HD:2 * HD],
                            s_sb[1][oc], None, AL.mult)
                dg1s.append(gt)
                bm = small.tile([HD, 1], f32, tag="bm", name=f"bm{oc}{sfx}")
                tmp = small.tile([HD, 1], f32, tag="bmt", name=None)
                nc.vector.tensor_scalar(tmp, bias1[0][oc], s_sb[0][oc],
                                        None, AL.mult)
                nc.vector.scalar_tensor_tensor(bm, bias1[1][oc], s_sb[1][oc],
                                               tmp, AL.mult, AL.add)
                bias_m.append(bm)

            # ---------------- conv1 (fused q+k -> m) ----------------
            for oc in range(NH):
                qplane = qk[oc]
                for t in range(NT):
                    y0 = 1 + TH * t
                    ps = mmp.tile([HD, FDC], f32, tag="mm",
                                  name=f"c1_{oc}_{t}{sfx}")
                    mms = []
                    for i in range(9):
                        grp, dx = divmod(i, 3)
                        dx -= 1
                        if grp == 0:    # q vertical pairs
                            mv = pairify(qplane,
                                         HEAD + (y0 - 1) * WP + dx,
                                         FDC, 2 * WP)
                        elif grp == 1:  # k vertical pairs
                            mv = pairify(qplane,
                                         BUFN + HEAD + (y0 - 1) * WP + dx,
                                         FDC, 2 * WP)
                        else:           # cross q/k middle row
                            mv = pairify(qplane, HEAD + y0 * WP + dx,
                                         FDC, BUFN)
                        st = dg1s[oc][:, i, :]
                        st = st.rearrange("p (two f) -> p two f", two=2)
                        mms.append((st, mv))
                    for i, (st, mv) in enumerate(mms):
                        nc.tensor.matmul(ps, st, mv, start=(i == 0),
                                         stop=(i == 8), perf_mode=DR)
                    nc.scalar.activation(
                        interior(m_t[oc], 0, t),
                        ps.rearrange("p (r c) -> p r c", c=WP)[:, :, 1:1 + W],
                        AF.Identity, bias=bias_m[oc], scale=1.0 / SCALE_W)

            # ---------------- conv2 + o2 ----------------
            for oc in range(NH):
                for t in range(NT):
                    y0 = 1 + TH * t
                    ps = mmp.tile([HD, FDC], f32, tag="mm",
                                  name=f"c2_{oc}_{t}{sfx}")
                    for i in range(3):
                        dx = i - 1
                        st = dg2p[oc][:, i, :]
                        st = st.rearrange("p (two f) -> p two f", two=2)
                        mv = pairify(m_t[oc], HEAD + (y0 - 1) * WP + dx,
                                     FDC, 2 * WP)
                        nc.tensor.matmul(ps, st, mv, start=(i == 0),
                                         stop=False, perf_mode=DR)
                    if cfg['conv2_midpair']:
                        st = dg2p[oc][:, 3, :]
                        st = st.rearrange("p (two f) -> p two f", two=2)
                        mv = pairify(m_t[oc], HEAD + y0 * WP - 1, FDC, 2)
                        nc.tensor.matmul(ps, st, mv, start=False, stop=False,
                                         perf_mode=DR)
                    else:
                        for pl, dx in ((0, -1), (1, 1)):
                            st = dg2p[oc][:, 3, pl * HD:(pl + 1) * HD]
                            nc.tensor.matmul(
                                ps, st,
                                m_t[oc][:, HEAD + y0 * WP + dx:
                                        HEAD + y0 * WP + dx + FDC],
                                start=False, stop=False)
                    nc.tensor.matmul(
                        ps, dg2s[oc],
                        m_t[oc][:, HEAD + y0 * WP:HEAD + y0 * WP + FDC],
                        start=False, stop=True)
                    c2t = c2pool.tile([HD, TN], f8, tag="c2t",
                                      name=f"c2t{oc}_{t}{sfx}")
                    nc.scalar.activation(
                        c2t.rearrange("p (r c) -> p r c", c=W),
                        ps.rearrange("p (r c) -> p r c", c=WP)[:, :, 1:1 + W],
                        AF.Identity, bias=dwcb[oc], scale=CSCALE / SCALE_W)
                    nc.vector.tensor_tensor(
                        o2_t[:, oc, t * TN:(t + 1) * TN], c2t,
                        v_t[:, oc, t * TN:(t + 1) * TN], AL.mult)

            # ---------------- proj + residual ----------------
            for ocp in range(NH):
                xf = xfs[ocp]
                for t in range(NT):
                    ps = mmp.tile([HD, TN], f32, tag="mm",
                                  name=f"p{ocp}_{t}{sfx}")
                    for kp in range(2):
                        nc.tensor.matmul(
                            ps,
                            wg["wp"][:, 2 * kp:2 * kp + 2,
                                     ocp * HD:(ocp + 1) * HD],
                            o2_t[:, 2 * kp:2 * kp + 2, t * TN:(t + 1) * TN],
                            start=(kp == 0), stop=(kp == 1), perf_mode=DR)
                    stg = stage.tile([HD, TN], f32, tag="stg",
                                     name=f"stg{ocp}_{t}{sfx}")
                    nc.vector.scalar_tensor_tensor(
                        stg, ps, 1.0 / (SCALE_W * CSCALE),
                        xf[:, t * TN:(t + 1) * TN], AL.mult, AL.add)
                    nc.sync.dma_start(
                        out_d[b, ocp * HD:(ocp + 1) * HD,
                              TH * t:TH * (t + 1), :],
                        stg.rearrange("p (r c) -> p r c", c=W))

        for b in range(BL):
            emit_sample(b)

    nc.compile()
    return nc


# ---------------------------------------------------------------------------
# host-side weight prep
# ---------------------------------------------------------------------------

def prep_weights(inputs, cfg):
    import ml_dtypes
    f8 = ml_dtypes.float8_e4m3
    f32 = np.float32

    qkv_w = np.asarray(inputs['qkv_w'], f32)
    proj_w = np.asarray(inputs['proj_w'], f32)

    def gemm_tile(wmat):
        # [HD p, NH kc, DIM oc*128+col] = SCALE_W * W[ocol, kc*128+p]
        arr = (SCALE_W * wmat).reshape(DIM, NH, HD).transpose(2, 1, 0)
        return np.ascontiguousarray(arr).astype(f8)

    wq = gemm_tile(qkv_w[0:DIM])
    wk = gemm_tile(qkv_w[DIM:2 * DIM])
    wv = gemm_tile(qkv_w[2 * DIM:3 * DIM])
    wp = gemm_tile(proj_w)

    sq = np.asarray(inputs['sq_w'], f32).reshape(DIM, 3, 3) * SCALE_W
    sk = np.asarray(inputs['sk_w'], f32).reshape(DIM, 3, 3) * SCALE_W
    dw = np.asarray(inputs['dwc_w'], f32).reshape(DIM, 3, 3) * SCALE_W

    idx = np.arange(HD)

    def diag(vals):
        d = np.zeros((HD, HD), f32)
        d[idx, idx] = vals
        return d

    # conv1 pair stationaries: [NH, 9, HD, 2*HD]
    dg1 = np.zeros((NH, 9, HD, 2 * HD), f32)
    for c in range(NH):
        q = sq[c * HD:(c + 1) * HD]
        k = sk[c * HD:(c + 1) * HD]
        for i in range(3):      # q vertical pairs, dx = i-1
            dg1[c, i, :, 0:HD] = diag(q[:, 0, i])
            dg1[c, i, :, HD:] = diag(q[:, 2, i])
        for i in range(3):      # k vertical pairs
            dg1[c, 3 + i, :, 0:HD] = diag(k[:, 0, i])
            dg1[c, 3 + i, :, HD:] = diag(k[:, 2, i])
        for i in range(3):      # cross middle row
            dg1[c, 6 + i, :, 0:HD] = diag(q[:, 1, i])
            dg1[c, 6 + i, :, HD:] = diag(k[:, 1, i])

    dg2p = np.zeros((NH, 4, HD, 2 * HD), f32)
    dg2s = np.zeros((NH, HD, HD), f32)
    for c in range(NH):
        d = dw[c * HD:(c + 1) * HD]
        for i in range(3):      # vertical pairs
            dg2p[c, i, :, 0:HD] = diag(d[:, 0, i])
            dg2p[c, i, :, HD:] = diag(d[:, 2, i])
        dg2p[c, 3, :, 0:HD] = diag(d[:, 1, 0])   # (0,-1)
        dg2p[c, 3, :, HD:] = diag(d[:, 1, 2])    # (0,+1)
        dg2s[c] = diag(d[:, 1, 1])

    # pooled-correction vectors (natural scale, /npix folded in):
    # pooled = A*S + B1*R0 + B2*R1 + B3*C0 + B4*C1 + c00*q00 + ...
    npix = float(H * W)
    pvv = np.zeros((2, DIM, 9), f32)
    # tap (dy,dx) reads rows [dy, 56+dy) clipped: dy=+1 excludes image row 0,
    # dy=-1 excludes row 55; dx likewise for cols.  Corner = doubly excluded.
    for br, wc in enumerate((sq / SCALE_W, sk / SCALE_W)):
        pvv[br, :, 0] = wc.sum(axis=(1, 2))
        pvv[br, :, 1] = -wc[:, 2, :].sum(axis=1)   # row 0 strip (dy=+1 taps)
        pvv[br, :, 2] = -wc[:, 0, :].sum(axis=1)   # row 55 strip (dy=-1)
        pvv[br, :, 3] = -wc[:, :, 2].sum(axis=1)   # col 0 strip (dx=+1)
        pvv[br, :, 4] = -wc[:, :, 0].sum(axis=1)   # col 55 strip (dx=-1)
        pvv[br, :, 5] = wc[:, 2, 2]                # q[0,0]
        pvv[br, :, 6] = wc[:, 2, 0]                # q[0,55]
        pvv[br, :, 7] = wc[:, 0, 2]                # q[55,0]
        pvv[br, :, 8] = wc[:, 0, 0]                # q[55,55]
    pvv /= npix

    sew1 = np.stack([
        np.asarray(inputs['cq_w1'], f32).transpose(0, 2, 1),
        np.asarray(inputs['ck_w1'], f32).transpose(0, 2, 1)])  # [2,NH,HD,HD4]
    seb1 = np.stack([np.asarray(inputs['cq_b1'], f32),
                     np.asarray(inputs['ck_b1'], f32)])        # [2,NH,HD4]
    sew2 = np.stack([
        np.asarray(inputs['cq_w2'], f32).transpose(0, 2, 1),
        np.asarray(inputs['ck_w2'], f32).transpose(0, 2, 1)])  # [2,NH,HD4,HD]
    seb2 = np.stack([np.asarray(inputs['cq_b2'], f32),
                     np.asarray(inputs['ck_b2'], f32)])        # [2,NH,HD]
    b1 = np.stack([np.asarray(inputs['sq_b'], f32),
                   np.asarray(inputs['sk_b'], f32)])           # [2,DIM]
    dwcb = CSCALE * np.asarray(inputs['dwc_b'], f32)           # [DIM]

    cpack = np.zeros((2, NH, HD, 44), f32)
    spack = np.zeros((2, NH, HD4, 129), f32)
    for br in range(2):
        for c in range(NH):
            sl = slice(c * HD, (c + 1) * HD)
            cpack[br, c, :, 0:9] = pvv[br, sl]
            cpack[br, c, :, 9:41] = sew1[br, c]
            cpack[br, c, :, 41] = seb2[br, c]
            cpack[br, c, :, 42] = b1[br, sl]
            spack[br, c, :, 0] = seb1[br, c]
            spack[br, c, :, 1:129] = sew2[br, c]
    for c in range(NH):
        cpack[0, c, :, 43] = dwcb[c * HD:(c + 1) * HD]

    return dict(
        wq=wq, wk=wk, wv=wv, wp=wp,
        dg1=dg1.astype(f8), dg2p=dg2p.astype(f8), dg2s=dg2s.astype(f8),
        cpack=cpack, spack=spack,
    )


_CACHE = {}


def _get_compiled(cfg_key, cfg):
    if cfg_key not in _CACHE:
        _CACHE[cfg_key] = build_nc(cfg)
    return _CACHE[cfg_key]


def make_in_maps(inputs, cfg):
    import ml_dtypes
    w = prep_weights(inputs, cfg)
    x32 = np.asarray(inputs['x'], np.float32)
    x8 = np.clip(x32, -240, 240).astype(ml_dtypes.float8_e4m3)
    projb = np.asarray(inputs['proj_b'], np.float32)
    xf = x32 + projb[None, :, None, None]
    BL = cfg['b_local']
    in_maps = []
    for core in range(N_CORES):
        mm = dict(w)
        mm['x8'] = np.ascontiguousarray(x8[core * BL:(core + 1) * BL])
        mm['xf'] = np.ascontiguousarray(xf[core * BL:(core + 1) * BL])
        in_maps.append(mm)
    return in_maps


def kernel(**inputs):
    from concourse import bass_utils
    cfg = default_cfg()
    nc = _get_compiled('main', cfg)
    in_maps = make_in_maps(inputs, cfg)
    res = bass_utils.run_bass_kernel_spmd(nc, in_maps,
                                          core_ids=list(range(N_CORES)))
    out = np.empty((B, DIM, H_FULL, W), np.float32)
    for core in range(N_CORES):
        out[core * BL:(core + 1) * BL] = res.results[core]['out']
    return out
